# revision 22
# baseline (speedup 1.0000x reference)
"""Trainium2 Bass kernel for nn_MinLSTMCell (B=8, T=4096, D=1024, H=1024).

Self-contained: hardcodes shapes/sharding. Data-parallel over batch B across
8 NeuronCores (one batch element per core).

Math (degenerate-cumsum form of the reference):
  h_t = f_{t-1} * S_t,  S_t = g(h0) + sum_{s<t} exp(diff_s) * g(zh_s),
  f = sigmoid(-diff), exp(diff) = (1+exp(-zf))*sigmoid(zi),
  g(z) = (1 + max(2z, tanh(z/2))) / 2.
Per element (exp/tanh act table, one table set):
  ef = exp(-(zf+bf)); ti = tanh((zi+bi)/2); th = tanh((zh+bh)/2)
  m2 = 2(zh+bh);  M = max(m2, th);  u2 = (ef+1)(ti+1) = 2*exp(diff)
  s  = cumsum((M+1)*u2) + 4*g(h0)   (fused multiply-scan custom DVE op)
  fq = 1/(u2+2) approx               (fused one-Newton reciprocal custom op)
  o  = s*fq = 2*h                    (host multiplies by 0.5)

Matmul precision (validated vs reference, absmax-norm err 0.0131 < 2e-2):
  zh: all 1024 channels fp8-e4m3 DoubleRow (0.5 col-cycles/row);
  zf/zi: first 256 channels fp8-DR, remaining 768 fp16 (1 cyc/row);
  all weights scaled by 4096 (fp8 representability), undone in act scale.
Elementwise fp16 (DVE 2x tensor_tensor, 4x tensor_scalar modes).
"""


import numpy as np
import ml_dtypes

import concourse.mybir as mybir
import concourse.tile as tile
from concourse import bacc
import concourse.dve_ops as _D
from concourse.dve_spec import Bin as _Bin, Spec as _Spec, Src0 as _S0, \
    Src1 as _S1, C0 as _C0, C1 as _C1, C2 as _C2, AluOp as _Alu, Scan as _Scan
from concourse.dve_ops import DveOp as _DveOp


def _reg_op(op):
    if not any(o.name == op.name for o in _D.OPS):
        _D.OPS.append(op)
        _D.CUSTOM_DVE_SPECS[op.name] = op.spec
        _D._SUB_OPCODE_FOR_NAME[op.name] = _D._CUSTOM_DVE_ROW_BASE + len(_D.OPS) - 1
    return op


def _ref_mulscan(in0, in1, c0, c1, c2):
    import numpy as _np
    prod = (in0.astype(_np.float32) + _np.float32(c2)) * in1.astype(_np.float32)
    return _np.cumsum(prod, axis=-1, dtype=_np.float32) + _np.asarray(
        c0, _np.float32)


def _ref_a2r(in0, in1, c0, c1, c2):
    import numpy as _np
    x = _np.ascontiguousarray(in0.astype(_np.float32)) + _np.float32(c2)
    y0 = ((~x.view(_np.int32)).view(_np.float32)) * _np.float32(c0)
    return y0 * (_np.float32(c1) - x * y0)


# out_t = cumsum((Src0 + imm2) * Src1) + s0   (s0 may be a [128,1] AP)
MULSCAN = _reg_op(_DveOp(
    "MULSCAN_ANT",
    _Spec(body=_Scan(_Alu.ADD, (_S0 + _C2) * _S1, init=_C0),
          reference=_ref_mulscan),
    subdim=False, uops_sha={"v3": "4090cc8eecca5b9f"},
))
# out = approx 1/(Src0 + imm2), one Newton pass (max rel err 0.17%
# for Src0+imm2 in [2, 36])
def _a2r_body():
    X = _Bin(_Alu.ADD, _S0, _C2)
    y0 = _Bin(_Alu.BITWISE_NOT, X, X) * _C0
    return y0 * (_C1 - X * y0)


A2RECIP = _reg_op(_DveOp(
    "ADD2RECIP1_ANT",
    _Spec(body=_a2r_body(), reference=_ref_a2r),
    subdim=False, uops_sha={"v3": "aa55afded45a0392"},
))
A2R_C0 = -0.23549794
A2R_C1 = 2.00173235


def _ref_fma11(in0, in1, c0, c1, c2):
    import numpy as _np
    return (in0.astype(_np.float32) + _np.float32(c0)) * (
        in1.astype(_np.float32) + _np.float32(c2))


# out = (Src0 + s0) * (Src1 + imm2)
FMA11 = _reg_op(_DveOp(
    "FMA11_ANT",
    _Spec(body=(_S0 + _C0) * (_S1 + _C2), reference=_ref_fma11),
    subdim=False, uops_sha={"v3": "66f7484353261fcc"},
))

B, T, D, H = 8, 4096, 1024, 1024
TB = 512            # t-block (psum free dim)
NTB = T // TB       # 8
NHT = H // 128      # 8 h-tiles
NDK = D // 128      # 8 d-chunks
NP = NDK // 2       # 4 fp8 pair-chunks
D8 = 256            # leading contraction channels of zf/zi done in fp8-DR
NK16 = (D - D8) // 128  # remaining fp16 chunks (6)
SW8 = 4096.0        # fp8 weight scale (power of 2); fp16 weights share it
F32 = mybir.dt.float32
F16 = mybir.dt.float16
F8 = mybir.dt.float8e4
AF = mybir.ActivationFunctionType
OP = mybir.AluOpType
DR = mybir.MatmulPerfMode.DoubleRow


def build_kernel():
    nc = bacc.Bacc()
    xt16 = nc.dram_tensor("xt16", [D - D8, T], F16, kind="ExternalInput")
    x8d = nc.dram_tensor("x8d", [128, NDK, T], F8, kind="ExternalInput")
    wf16d = nc.dram_tensor("wf16", [D - D8, H], F16, kind="ExternalInput")
    wi16d = nc.dram_tensor("wi16", [D - D8, H], F16, kind="ExternalInput")
    wf8d = nc.dram_tensor("wf8", [128, D8 // 256, 2, H], F8, kind="ExternalInput")
    wi8d = nc.dram_tensor("wi8", [128, D8 // 256, 2, H], F8, kind="ExternalInput")
    wh8d = nc.dram_tensor("wh8", [128, NP, 2, H], F8, kind="ExternalInput")
    nbf = nc.dram_tensor("nbf", [128, NHT], F32, kind="ExternalInput")  # -bf
    hbi = nc.dram_tensor("hbi", [128, NHT], F32, kind="ExternalInput")  # bi/2
    hbh = nc.dram_tensor("hbh", [128, NHT], F32, kind="ExternalInput")  # bh/2
    b2h = nc.dram_tensor("b2h", [128, NHT], F32, kind="ExternalInput")  # 2*bh
    g4c = nc.dram_tensor("g4c", [128, NHT], F32, kind="ExternalInput")  # 4*g0
    out16 = nc.dram_tensor("out16", [H, T], F16, kind="ExternalOutput")

    with tile.TileContext(nc) as tc:
        with (
            tc.tile_pool(name="singles", bufs=1) as singles,
            tc.tile_pool(name="xfp", bufs=18) as xf_p,
            tc.tile_pool(name="x8p", bufs=2) as x8_p,
            tc.tile_pool(name="pz", bufs=6, space="PSUM") as pz,
            tc.tile_pool(name="ew", bufs=6) as ew,
            tc.tile_pool(name="scan", bufs=9) as scan_p,
        ):
            def emit_xload(tb):
                t0 = tb * TB
                tiles = []
                for k in range(NK16):
                    xk = xf_p.tile([128, TB], F16, tag="xT")
                    nc.sync.dma_start(xk[:], xt16[k * 128:(k + 1) * 128, t0:t0 + TB])
                    tiles.append(xk)
                x8t = x8_p.tile([128, NDK, TB], F8, tag="x8")
                for j in range(NP):
                    eng = nc.scalar if j % 2 == 0 else nc.sync
                    eng.dma_start(
                        x8t[:, 2 * j:2 * j + 2, :],
                        x8d[:, 2 * j:2 * j + 2, t0:t0 + TB],
                    )
                return tiles, x8t

            # x for tb0 loads first, then weights (fp8 ones first: the
            # first matmuls of each gate need them)
            x16_cur, x8_cur = emit_xload(0)
            wf8_sb = singles.tile([128, 2, H], F8, tag="Wf8")
            nc.sync.dma_start(wf8_sb[:], wf8d[:, 0, :, :])
            wi8_sb = singles.tile([128, 2, H], F8, tag="Wi8")
            nc.scalar.dma_start(wi8_sb[:], wi8d[:, 0, :, :])
            wh_sb = []
            for j in range(NP):
                th = singles.tile([128, 2, H], F8, tag=f"Wh{j}")
                eng = nc.scalar if j % 2 == 0 else nc.sync
                eng.dma_start(th[:], wh8d[:, j, :, :])
                wh_sb.append(th)
            wf_sb, wi_sb = [], []
            for k in range(NK16):
                tf = singles.tile([128, H], F16, tag=f"Wf{k}")
                eng = nc.scalar if k % 2 == 0 else nc.sync
                eng.dma_start(tf[:], wf16d[k * 128:(k + 1) * 128, :])
                wf_sb.append(tf)
                ti_ = singles.tile([128, H], F16, tag=f"Wi{k}")
                eng = nc.sync if k % 2 == 0 else nc.scalar
                eng.dma_start(ti_[:], wi16d[k * 128:(k + 1) * 128, :])
                wi_sb.append(ti_)
            bias = {}
            for name, dr in (("nbf", nbf), ("hbi", hbi), ("hbh", hbh),
                             ("b2h", b2h), ("g4", g4c)):
                t = singles.tile([128, NHT], F32, tag=name)
                nc.sync.dma_start(t[:], dr[:])
                bias[name] = t

            s_prev = [None] * NHT
            for tb in range(NTB):
                t0 = tb * TB
                x16, x8t = x16_cur, x8_cur
                for ht in range(NHT):
                    hs = slice(ht * 128, (ht + 1) * 128)
                    zf = pz.tile([128, TB], F32, tag="z")
                    for hh in range(2):
                        cs = slice(hh * 256, (hh + 1) * 256)
                        nc.tensor.matmul(
                            zf[:, cs], wf8_sb[:, :, hs], x8t[:, 0:2, cs],
                            start=True, stop=False, perf_mode=DR,
                        )
                        for k in range(NK16):
                            nc.tensor.matmul(
                                zf[:, cs], wf_sb[k][:, hs], x16[k][:, cs],
                                start=False, stop=(k == NK16 - 1),
                            )
                    zi = pz.tile([128, TB], F32, tag="z")
                    for hh in range(2):
                        cs = slice(hh * 256, (hh + 1) * 256)
                        nc.tensor.matmul(
                            zi[:, cs], wi8_sb[:, :, hs], x8t[:, 0:2, cs],
                            start=True, stop=False, perf_mode=DR,
                        )
                        for k in range(NK16):
                            nc.tensor.matmul(
                                zi[:, cs], wi_sb[k][:, hs], x16[k][:, cs],
                                start=False, stop=(k == NK16 - 1),
                            )
                    zh = pz.tile([128, TB], F32, tag="z")
                    for hh in range(2):
                        cs = slice(hh * 256, (hh + 1) * 256)
                        for j in range(NP):
                            nc.tensor.matmul(
                                zh[:, cs],
                                wh_sb[j][:, :, hs],
                                x8t[:, 2 * j:2 * j + 2, cs],
                                start=(j == 0), stop=(j == NP - 1),
                                perf_mode=DR,
                            )
                    # prefetch next block's x
                    if tb + 1 < NTB and ht == 0:
                        x16_cur, x8_cur = emit_xload(tb + 1)
                    # ---- ACT phase (exp table set: Exp/Tanh/Identity)
                    ef = ew.tile([128, TB], F16, tag="ef")
                    nc.scalar.activation(
                        ef[:], zf[:], AF.Exp,
                        bias=bias["nbf"][:, ht:ht + 1], scale=-1.0 / SW8)
                    ti = ew.tile([128, TB], F16, tag="ti")
                    nc.scalar.activation(
                        ti[:], zi[:], AF.Tanh,
                        bias=bias["hbi"][:, ht:ht + 1], scale=0.5 / SW8)
                    th = ew.tile([128, TB], F16, tag="th")
                    nc.scalar.activation(
                        th[:], zh[:], AF.Tanh,
                        bias=bias["hbh"][:, ht:ht + 1], scale=0.5 / SW8)
                    m2 = ew.tile([128, TB], F16, tag="m2")
                    nc.scalar.activation(
                        m2[:], zh[:], AF.Identity,
                        bias=bias["b2h"][:, ht:ht + 1], scale=2.0 / SW8)
                    # ---- DVE phase (fp16 SBUF; ts 4x / tt 2x / stt+scan 1x)
                    M = ew.tile([128, TB], F16, tag="M")
                    nc.vector.tensor_tensor(M[:], m2[:], th[:], op=OP.max)
                    u2 = ew.tile([128, TB], F16, tag="u2")
                    nc.vector._custom_dve(
                        FMA11, out=u2[:], in0=ef[:], in1=ti[:],
                        s0=1.0, imm2=1.0)
                    # fq first: o depends on (s_t, fq); fq frees earlier
                    fq = ew.tile([128, TB], F16, tag="fq")
                    nc.vector._custom_dve(
                        A2RECIP, out=fq[:], in0=u2[:],
                        s0=A2R_C0, s1=A2R_C1, imm2=2.0)
                    s_t = scan_p.tile([128, TB], F32, tag="S")
                    init = (
                        bias["g4"][:, ht:ht + 1] if tb == 0
                        else s_prev[ht][:, TB - 1:TB]
                    )
                    # s_t = cumsum((M+1)*u2) + init  (= 4*S running sum)
                    nc.vector._custom_dve(
                        MULSCAN, out=s_t[:], in0=M[:], in1=u2[:],
                        s0=init, imm2=1.0)
                    s_prev[ht] = s_t
                    o = ew.tile([128, TB], F16, tag="o")
                    nc.vector.tensor_tensor(o[:], s_t[:], fq[:], op=OP.mult)
                    oeng = nc.sync if ht % 2 == 0 else nc.scalar
                    oeng.dma_start(out16[hs, t0:t0 + TB], o[:])
    nc.finalize()
    return nc


_NC_CACHE = None


def get_nc():
    global _NC_CACHE
    if _NC_CACHE is None:
        _NC_CACHE = build_kernel()
    return _NC_CACHE


def kernel(x_t, h_prev, Wf, bf, Wi, bi, Wh, bh, _run_opts=None):
    from concourse.bass_utils import run_bass_kernel_spmd

    x_t = np.asarray(x_t, dtype=np.float32)
    h_prev = np.asarray(h_prev, dtype=np.float32)
    Wf = np.asarray(Wf, dtype=np.float32)
    Wi = np.asarray(Wi, dtype=np.float32)
    Wh = np.asarray(Wh, dtype=np.float32)
    bf = np.asarray(bf, dtype=np.float32)
    bi = np.asarray(bi, dtype=np.float32)
    bh = np.asarray(bh, dtype=np.float32)

    nc = get_nc()

    g0 = np.maximum(h_prev + 0.5, 1.0 / (1.0 + np.exp(-h_prev))).astype(np.float32)
    colmaj = lambda v: np.ascontiguousarray(
        np.asarray(v, np.float32).reshape(NHT, 128).T)
    nbf = colmaj(-bf)
    hbi = colmaj(0.5 * bi)
    hbh = colmaj(0.5 * bh)
    b2h = colmaj(2.0 * bh)

    wf16 = np.ascontiguousarray((Wf[D8:] * SW8).astype(np.float16))
    wi16 = np.ascontiguousarray((Wi[D8:] * SW8).astype(np.float16))
    q8w = lambda W: np.ascontiguousarray(
        (W[:D8] * SW8).reshape(D8 // 256, 2, 128, H).transpose(2, 0, 1, 3)
        .astype(ml_dtypes.float8_e4m3fn))
    wf8 = q8w(Wf)
    wi8 = q8w(Wi)
    # wh8[k, j, i, h] = q8(Wh[(2j+i)*128+k, h] * SW8)
    wh8 = np.ascontiguousarray(
        (Wh * SW8).reshape(NP, 2, 128, H).transpose(2, 0, 1, 3)
        .astype(ml_dtypes.float8_e4m3fn))

    in_maps = []
    for b in range(B):
        xT = x_t[b].T  # [D, T]
        xt16 = np.ascontiguousarray(xT[D8:].astype(np.float16))
        x8 = np.ascontiguousarray(
            xT.reshape(NDK, 128, T).transpose(1, 0, 2)
            .astype(ml_dtypes.float8_e4m3fn))
        in_maps.append({
            "xt16": xt16, "x8d": x8,
            "wf16": wf16, "wi16": wi16, "wh8": wh8,
            "wf8": wf8, "wi8": wi8,
            "nbf": nbf, "hbi": hbi, "hbh": hbh, "b2h": b2h,
            "g4c": colmaj(4.0 * g0[b]),
        })

    opts = _run_opts or {}
    res = run_bass_kernel_spmd(nc, in_maps, core_ids=list(range(B)), **opts)

    out = np.empty((B, T + 1, H), dtype=np.float32)
    for b in range(B):
        out[b, 0, :] = g0[b]
        out[b, 1:, :] = 0.5 * res.results[b]["out16"].astype(np.float32).T
    if _run_opts is not None:
        return out, res
    return out


# revision 23
# speedup vs baseline: 1.1904x; 1.1904x over previous
"""Trainium2 Bass kernel for nn_MinLSTMCell (B=8, T=4096, D=1024, H=1024).

Self-contained: hardcodes shapes/sharding. Data-parallel over batch B across
8 NeuronCores (one batch element per core).

Math (degenerate-cumsum form of the reference):
  h_t = f_{t-1} * S_t,  S_t = g(h0) + sum_{s<t} exp(diff_s) * g(zh_s),
  f = sigmoid(-diff), exp(diff) = (1+exp(-zf))*sigmoid(zi),
  g(z) = (1 + max(2z, tanh(z/2))) / 2.
Per element (exp/tanh act table, one table set):
  ef = exp(-(zf+bf)); ti = tanh((zi+bi)/2); th = tanh((zh+bh)/2)
  m2 = 2(zh+bh);  M = max(m2, th);  u2 = (ef+1)(ti+1) = 2*exp(diff)
  s  = cumsum((M+1)*u2) + 4*g(h0)   (fused multiply-scan custom DVE op)
  fq = 1/(u2+2) approx               (fused one-Newton reciprocal custom op)
  o  = s*fq = 2*h                    (host multiplies by 0.5)

Matmul precision (validated vs reference, absmax-norm err 0.0131 < 2e-2):
  zh: all 1024 channels fp8-e4m3 DoubleRow (0.5 col-cycles/row);
  zf/zi: first 256 channels fp8-DR, remaining 768 fp16 (1 cyc/row);
  all weights scaled by 4096 (fp8 representability), undone in act scale.
Elementwise fp16 (DVE 2x tensor_tensor, 4x tensor_scalar modes).
"""


import numpy as np
import ml_dtypes

import concourse.mybir as mybir
import concourse.tile as tile
from concourse import bacc
import concourse.dve_ops as _D
from concourse.dve_spec import Bin as _Bin, Spec as _Spec, Src0 as _S0, \
    Src1 as _S1, C0 as _C0, C1 as _C1, C2 as _C2, AluOp as _Alu, Scan as _Scan
from concourse.dve_ops import DveOp as _DveOp


def _reg_op(op):
    if not any(o.name == op.name for o in _D.OPS):
        _D.OPS.append(op)
        _D.CUSTOM_DVE_SPECS[op.name] = op.spec
        _D._SUB_OPCODE_FOR_NAME[op.name] = _D._CUSTOM_DVE_ROW_BASE + len(_D.OPS) - 1
    return op


def _ref_mulscan(in0, in1, c0, c1, c2):
    import numpy as _np
    prod = (in0.astype(_np.float32) + _np.float32(c2)) * in1.astype(_np.float32)
    return _np.cumsum(prod, axis=-1, dtype=_np.float32) + _np.asarray(
        c0, _np.float32)


def _ref_a2r(in0, in1, c0, c1, c2):
    import numpy as _np
    x = _np.ascontiguousarray(in0.astype(_np.float32)) + _np.float32(c2)
    y0 = ((~x.view(_np.int32)).view(_np.float32)) * _np.float32(c0)
    return y0 * (_np.float32(c1) - x * y0)


# out_t = cumsum((Src0 + imm2) * Src1) + s0   (s0 may be a [128,1] AP)
MULSCAN = _reg_op(_DveOp(
    "MULSCAN_ANT",
    _Spec(body=_Scan(_Alu.ADD, (_S0 + _C2) * _S1, init=_C0),
          reference=_ref_mulscan),
    subdim=False, uops_sha={"v3": "4090cc8eecca5b9f"},
))
# out = approx 1/(Src0 + imm2), one Newton pass (max rel err 0.17%
# for Src0+imm2 in [2, 36])
def _a2r_body():
    X = _Bin(_Alu.ADD, _S0, _C2)
    y0 = _Bin(_Alu.BITWISE_NOT, X, X) * _C0
    return y0 * (_C1 - X * y0)


A2RECIP = _reg_op(_DveOp(
    "ADD2RECIP1_ANT",
    _Spec(body=_a2r_body(), reference=_ref_a2r),
    subdim=False, uops_sha={"v3": "aa55afded45a0392"},
))
A2R_C0 = -0.23549794
A2R_C1 = 2.00173235


def _ref_fma11(in0, in1, c0, c1, c2):
    import numpy as _np
    return (in0.astype(_np.float32) + _np.float32(c0)) * (
        in1.astype(_np.float32) + _np.float32(c2))


# out = (Src0 + s0) * (Src1 + imm2)
FMA11 = _reg_op(_DveOp(
    "FMA11_ANT",
    _Spec(body=(_S0 + _C0) * (_S1 + _C2), reference=_ref_fma11),
    subdim=False, uops_sha={"v3": "66f7484353261fcc"},
))

B, T, D, H = 8, 4096, 1024, 1024
TB = 512            # t-block (psum free dim)
NTB = T // TB       # 8
NHT = H // 128      # 8 h-tiles
NDK = D // 128      # 8 d-chunks
NP = NDK // 2       # 4 fp8 pair-chunks
D8 = 256            # leading contraction channels of zf/zi done in fp8-DR
NK16 = (D - D8) // 128  # remaining fp16 chunks (6)
SW8 = 4096.0        # fp8 weight scale (power of 2); fp16 weights share it
F32 = mybir.dt.float32
F16 = mybir.dt.float16
F8 = mybir.dt.float8e4
AF = mybir.ActivationFunctionType
OP = mybir.AluOpType
DR = mybir.MatmulPerfMode.DoubleRow


def build_kernel():
    nc = bacc.Bacc()
    xt16 = nc.dram_tensor("xt16", [D - D8, T], F16, kind="ExternalInput")
    x8d = nc.dram_tensor("x8d", [128, NDK, T], F8, kind="ExternalInput")
    wf16d = nc.dram_tensor("wf16", [D - D8, H], F16, kind="ExternalInput")
    wi16d = nc.dram_tensor("wi16", [D - D8, H], F16, kind="ExternalInput")
    wf8d = nc.dram_tensor("wf8", [128, D8 // 256, 2, H], F8, kind="ExternalInput")
    wi8d = nc.dram_tensor("wi8", [128, D8 // 256, 2, H], F8, kind="ExternalInput")
    wh8d = nc.dram_tensor("wh8", [128, NP, 2, H], F8, kind="ExternalInput")
    nbf = nc.dram_tensor("nbf", [128, NHT], F32, kind="ExternalInput")  # -bf
    hbi = nc.dram_tensor("hbi", [128, NHT], F32, kind="ExternalInput")  # bi/2
    hbh = nc.dram_tensor("hbh", [128, NHT], F32, kind="ExternalInput")  # bh/2
    b2h = nc.dram_tensor("b2h", [128, NHT], F32, kind="ExternalInput")  # 2*bh
    g4c = nc.dram_tensor("g4c", [128, NHT], F32, kind="ExternalInput")  # 4*g0
    out16 = nc.dram_tensor("out16", [H, T], F16, kind="ExternalOutput")

    with tile.TileContext(nc) as tc:
        with (
            tc.tile_pool(name="singles", bufs=1) as singles,
            tc.tile_pool(name="xfp", bufs=18) as xf_p,
            tc.tile_pool(name="x8p", bufs=2) as x8_p,
            tc.tile_pool(name="pz", bufs=6, space="PSUM") as pz,
            tc.tile_pool(name="ew", bufs=6) as ew,
            tc.tile_pool(name="scan", bufs=9) as scan_p,
        ):
            def emit_xload(tb):
                t0 = tb * TB
                tiles = []
                for k in range(NK16):
                    xk = xf_p.tile([128, TB], F16, tag="xT")
                    nc.sync.dma_start(xk[:], xt16[k * 128:(k + 1) * 128, t0:t0 + TB])
                    tiles.append(xk)
                x8t = x8_p.tile([128, NDK, TB], F8, tag="x8")
                for j in range(NP):
                    eng = nc.scalar if j % 2 == 0 else nc.sync
                    eng.dma_start(
                        x8t[:, 2 * j:2 * j + 2, :],
                        x8d[:, 2 * j:2 * j + 2, t0:t0 + TB],
                    )
                return tiles, x8t

            # x for tb0 loads first, then weights (fp8 ones first: the
            # first matmuls of each gate need them)
            x16_cur, x8_cur = emit_xload(0)
            wf8_sb = singles.tile([128, 2, H], F8, tag="Wf8")
            nc.sync.dma_start(wf8_sb[:], wf8d[:, 0, :, :])
            wi8_sb = singles.tile([128, 2, H], F8, tag="Wi8")
            nc.scalar.dma_start(wi8_sb[:], wi8d[:, 0, :, :])
            wh_sb = []
            for j in range(NP):
                th = singles.tile([128, 2, H], F8, tag=f"Wh{j}")
                eng = nc.scalar if j % 2 == 0 else nc.sync
                eng.dma_start(th[:], wh8d[:, j, :, :])
                wh_sb.append(th)
            wf_sb, wi_sb = [], []
            for k in range(NK16):
                tf = singles.tile([128, H], F16, tag=f"Wf{k}")
                eng = nc.scalar if k % 2 == 0 else nc.sync
                eng.dma_start(tf[:], wf16d[k * 128:(k + 1) * 128, :])
                wf_sb.append(tf)
                ti_ = singles.tile([128, H], F16, tag=f"Wi{k}")
                eng = nc.sync if k % 2 == 0 else nc.scalar
                eng.dma_start(ti_[:], wi16d[k * 128:(k + 1) * 128, :])
                wi_sb.append(ti_)
            bias = {}
            for name, dr in (("nbf", nbf), ("hbi", hbi), ("hbh", hbh),
                             ("b2h", b2h), ("g4", g4c)):
                t = singles.tile([128, NHT], F32, tag=name)
                nc.sync.dma_start(t[:], dr[:])
                bias[name] = t

            s_prev = [None] * NHT
            for tb in range(NTB):
                t0 = tb * TB
                x16, x8t = x16_cur, x8_cur
                for ht in range(NHT):
                    hs = slice(ht * 128, (ht + 1) * 128)
                    zf = pz.tile([128, TB], F32, tag="z")
                    for hh in range(2):
                        cs = slice(hh * 256, (hh + 1) * 256)
                        nc.tensor.matmul(
                            zf[:, cs], wf8_sb[:, :, hs], x8t[:, 0:2, cs],
                            start=True, stop=False, perf_mode=DR,
                        )
                        for k in range(NK16):
                            nc.tensor.matmul(
                                zf[:, cs], wf_sb[k][:, hs], x16[k][:, cs],
                                start=False, stop=(k == NK16 - 1),
                            )
                    zi = pz.tile([128, TB], F32, tag="z")
                    for hh in range(2):
                        cs = slice(hh * 256, (hh + 1) * 256)
                        nc.tensor.matmul(
                            zi[:, cs], wi8_sb[:, :, hs], x8t[:, 0:2, cs],
                            start=True, stop=False, perf_mode=DR,
                        )
                        for k in range(NK16):
                            nc.tensor.matmul(
                                zi[:, cs], wi_sb[k][:, hs], x16[k][:, cs],
                                start=False, stop=(k == NK16 - 1),
                            )
                    zh = pz.tile([128, TB], F32, tag="z")
                    for hh in range(2):
                        cs = slice(hh * 256, (hh + 1) * 256)
                        for j in range(NP):
                            nc.tensor.matmul(
                                zh[:, cs],
                                wh_sb[j][:, :, hs],
                                x8t[:, 2 * j:2 * j + 2, cs],
                                start=(j == 0), stop=(j == NP - 1),
                                perf_mode=DR,
                            )
                    # prefetch next block's x
                    if tb + 1 < NTB and ht == 0:
                        x16_cur, x8_cur = emit_xload(tb + 1)
                    # ---- ACT phase (exp table set: Exp/Tanh/Identity)
                    ef = ew.tile([128, TB], F16, tag="ef")
                    nc.scalar.activation(
                        ef[:], zf[:], AF.Exp,
                        bias=bias["nbf"][:, ht:ht + 1], scale=-1.0 / SW8)
                    ti = ew.tile([128, TB], F16, tag="ti")
                    nc.scalar.activation(
                        ti[:], zi[:], AF.Tanh,
                        bias=bias["hbi"][:, ht:ht + 1], scale=0.5 / SW8)
                    th = ew.tile([128, TB], F16, tag="th")
                    nc.scalar.activation(
                        th[:], zh[:], AF.Tanh,
                        bias=bias["hbh"][:, ht:ht + 1], scale=0.5 / SW8)
                    m2 = ew.tile([128, TB], F16, tag="m2")
                    nc.scalar.activation(
                        m2[:], zh[:], AF.Identity,
                        bias=bias["b2h"][:, ht:ht + 1], scale=2.0 / SW8)
                    # ---- DVE phase (fp16 SBUF; ts 4x / tt 2x / stt+scan 1x)
                    M = ew.tile([128, TB], F16, tag="M")
                    nc.vector.tensor_tensor(M[:], m2[:], th[:], op=OP.max)
                    u2 = ew.tile([128, TB], F16, tag="u2")
                    nc.vector._custom_dve(
                        FMA11, out=u2[:], in0=ef[:], in1=ti[:],
                        s0=1.0, imm2=1.0)
                    # fq first: o depends on (s_t, fq); fq frees earlier
                    fq = ew.tile([128, TB], F16, tag="fq")
                    nc.vector._custom_dve(
                        A2RECIP, out=fq[:], in0=u2[:],
                        s0=A2R_C0, s1=A2R_C1, imm2=2.0)
                    s_t = scan_p.tile([128, TB], F32, tag="S")
                    init = (
                        bias["g4"][:, ht:ht + 1] if tb == 0
                        else s_prev[ht][:, TB - 1:TB]
                    )
                    # s_t = cumsum((M+1)*u2) + init  (= 4*S running sum)
                    nc.vector._custom_dve(
                        MULSCAN, out=s_t[:], in0=M[:], in1=u2[:],
                        s0=init, imm2=1.0)
                    s_prev[ht] = s_t
                    o = ew.tile([128, TB], F16, tag="o")
                    nc.vector.tensor_tensor(o[:], s_t[:], fq[:], op=OP.mult)
                    nc.sync.dma_start(out16[hs, t0:t0 + TB], o[:])
    nc.finalize()
    return nc


_NC_CACHE = None


def get_nc():
    global _NC_CACHE
    if _NC_CACHE is None:
        _NC_CACHE = build_kernel()
    return _NC_CACHE


def kernel(x_t, h_prev, Wf, bf, Wi, bi, Wh, bh, _run_opts=None):
    from concourse.bass_utils import run_bass_kernel_spmd

    x_t = np.asarray(x_t, dtype=np.float32)
    h_prev = np.asarray(h_prev, dtype=np.float32)
    Wf = np.asarray(Wf, dtype=np.float32)
    Wi = np.asarray(Wi, dtype=np.float32)
    Wh = np.asarray(Wh, dtype=np.float32)
    bf = np.asarray(bf, dtype=np.float32)
    bi = np.asarray(bi, dtype=np.float32)
    bh = np.asarray(bh, dtype=np.float32)

    nc = get_nc()

    g0 = np.maximum(h_prev + 0.5, 1.0 / (1.0 + np.exp(-h_prev))).astype(np.float32)
    colmaj = lambda v: np.ascontiguousarray(
        np.asarray(v, np.float32).reshape(NHT, 128).T)
    nbf = colmaj(-bf)
    hbi = colmaj(0.5 * bi)
    hbh = colmaj(0.5 * bh)
    b2h = colmaj(2.0 * bh)

    wf16 = np.ascontiguousarray((Wf[D8:] * SW8).astype(np.float16))
    wi16 = np.ascontiguousarray((Wi[D8:] * SW8).astype(np.float16))
    q8w = lambda W: np.ascontiguousarray(
        (W[:D8] * SW8).reshape(D8 // 256, 2, 128, H).transpose(2, 0, 1, 3)
        .astype(ml_dtypes.float8_e4m3fn))
    wf8 = q8w(Wf)
    wi8 = q8w(Wi)
    # wh8[k, j, i, h] = q8(Wh[(2j+i)*128+k, h] * SW8)
    wh8 = np.ascontiguousarray(
        (Wh * SW8).reshape(NP, 2, 128, H).transpose(2, 0, 1, 3)
        .astype(ml_dtypes.float8_e4m3fn))

    in_maps = []
    for b in range(B):
        xT = x_t[b].T  # [D, T]
        xt16 = np.ascontiguousarray(xT[D8:].astype(np.float16))
        x8 = np.ascontiguousarray(
            xT.reshape(NDK, 128, T).transpose(1, 0, 2)
            .astype(ml_dtypes.float8_e4m3fn))
        in_maps.append({
            "xt16": xt16, "x8d": x8,
            "wf16": wf16, "wi16": wi16, "wh8": wh8,
            "wf8": wf8, "wi8": wi8,
            "nbf": nbf, "hbi": hbi, "hbh": hbh, "b2h": b2h,
            "g4c": colmaj(4.0 * g0[b]),
        })

    opts = _run_opts or {}
    res = run_bass_kernel_spmd(nc, in_maps, core_ids=list(range(B)), **opts)

    out = np.empty((B, T + 1, H), dtype=np.float32)
    for b in range(B):
        out[b, 0, :] = g0[b]
        out[b, 1:, :] = 0.5 * res.results[b]["out16"].astype(np.float32).T
    if _run_opts is not None:
        return out, res
    return out
